# revision 32
# baseline (speedup 1.0000x reference)
"""BlockSparse Ring Multihead Dilated Attention — Trainium2 Bass kernel.

Decomposition: the LongNet-style dilated attention factors into 28 independent
dense 2048x2048 attention "units" (one per head x segment x dilation-offset).
Each of the 8 cores gets a perfectly balanced bundle:
  set A: one group-0 pair   (2 heads, same 2048-token segment, 2048 queries)
  set B: one group-1 pair   (2 heads, same dilated 2048-token set, 1024 queries)
  set C: one group-2 head   (1 head, dilated 2048-token set, 1024 queries)
Inputs are pre-gathered/transposed/bf16-cast on the host (that's the shard
step); each core runs QKV projection, attention (scores^T layout, softmax
denominator via a ones-column appended to V), and the output projection for
its (token, head) slice. The host scatter-adds the 8 partial out-projections
(tensor-parallel unshard) and adds the output bias.

v2 (217us -> target ~160us): the score matmuls contract over head_dim=64 —
half the 128-row PE array. They are now issued as row-tiled concurrent PAIRS
(bass auto-derives tile_position from the lhsT/rhs base partition):
  - 2-head sets: head a lives on SBUF partitions 0-63, head b on 64-127, so
    interleaving the emission (a_kt, b_kt, a_kt+1, b_kt+1) makes each pair
    run concurrently on disjoint row groups (~2x score throughput).
  - set C (1 head): the host packs its q/k weight rows TWICE (rows 64-127 of
    qT/kT duplicate 0-63; matmul cost is N-bound so the duplication is free),
    letting the two kt-chunks of a slot pair on row groups h0/h64.
The per-slot emission order of the two heads alternates so the PSUM st-buffer
rotation always reuses the buffer whose exp finished first (the old fixed
order cost ~1.2us/slot in PE waits).  Warmup matmuls run off a memset tile
(no DMA dependency) sized to bridge the input-DMA window so the HAM clock
governor is at full rate when real work starts; the C tail interleaves a few
keep-warm matmuls so the final recip/norm/outproj never opens a >3.4us PE gap
(the old kernel's tail ran ~21us at half clock).  Input DMAs are split across
both HWDGE rings (sync + scalar engines).

Hardware notes baked into the structure: PE clock throttles to 1/2 after
~3.4us of PE idle (HAM); gpsimd cannot access PSUM; DVE/activation ops need
0/32/64/96-aligned start partitions; exp costs (N+352)/1.2 ns on the scalar
engine and is the secondary bottleneck (~128us/core) — scores must arrive in
PSUM early enough that ACT never starves; fp8 (e4m3) for scores or P@V was
measured at ~2.3% final error vs the 2% tolerance and reverted.
"""

import numpy as np
import ml_dtypes

BF16 = ml_dtypes.bfloat16

# wbuf column offsets (bf16 [128, 8832])
_WOFF = {
    "A": {"q": 0, "k": 768, "v": 1536, "o": 2304},
    "B": {"q": 3072, "k": 3840, "v": 4608, "o": 5376},
    "C": {"q": 6144, "k": 6912, "v": 7680, "o": 8064},
}
# bbuf column indices (f32 [128, 9])
_BOFF = {
    "A": {"q": 0, "k": 1, "v": 2},
    "B": {"q": 3, "k": 4, "v": 5},
    "C": {"q": 6, "k": 7, "v": 8},
}

_CACHE = {}


def _core_plan(c):
    """Unit assignment for core c (0..7)."""
    # set A: group-0 (seg 2048, r=1): seg = c//2, heads (0,1) or (2,3)
    segA = c // 2
    haA = 2 * (c % 2)
    orderA = segA * 2048 + np.arange(2048)
    # set B: group-1 (seg 4096, r=2): pairs (4,6) parity0 / (5,7) parity1
    seg1 = c // 4
    p = (c % 4) // 2
    qh_b = c % 2
    hB = (4 + p, 6 + p)
    tokB = seg1 * 4096 + p + 2 * np.arange(2048)
    orderB = np.concatenate([tokB[qh_b * 1024:(qh_b + 1) * 1024],
                             tokB[(1 - qh_b) * 1024:(2 - qh_b) * 1024]])
    # set C: group-2 (seg 8192, r=4): head 8+j owns tokens j + 4*arange
    j = c // 2
    qh_c = c % 2
    hC = 8 + j
    tokC = j + 4 * np.arange(2048)
    orderC = np.concatenate([tokC[qh_c * 1024:(qh_c + 1) * 1024],
                             tokC[(1 - qh_c) * 1024:(2 - qh_c) * 1024]])
    return {
        "A": {"heads": (haA, haA + 1), "order": orderA},
        "B": {"heads": hB, "order": orderB},
        "C": {"heads": (hC,), "order": orderC},
    }


def _pack_lhsT(w_rows):
    """[M, 768] weight rows -> [128, 6*M] bf16 (e-chunked lhsT layout)."""
    m = w_rows.shape[0]
    t = w_rows.T.reshape(6, 128, m).transpose(1, 0, 2)  # [128, 6, M]
    return np.ascontiguousarray(t.reshape(128, 6 * m)).astype(BF16)


def _prep_core_inputs(c, x, qkv_w, qkv_b, out_w):
    plan = _core_plan(c)
    x2 = x[0]  # [8192, 768] f32
    ins = {}
    wcols = []
    bcols = np.zeros((128, 9), np.float32)
    for s in "ABC":
        heads = plan[s]["heads"]
        order = plan[s]["order"]
        xs = x2[order]  # [2048, 768]
        xt = xs.T.reshape(6, 128, 2048).transpose(1, 0, 2)  # [128, 6, 2048]
        # chunk-major layout: each 512-token chunk is contiguous per
        # partition (6KB runs) so the chunk DMAs hit full HBM throughput.
        xt4 = xt.reshape(128, 6, 4, 512).transpose(2, 0, 1, 3)
        ins[f"xt{s}"] = np.ascontiguousarray(xt4).astype(BF16)
        qrows = np.concatenate([qkv_w[h * 64:(h + 1) * 64] for h in heads], 0)
        krows = np.concatenate([qkv_w[768 + h * 64:768 + (h + 1) * 64] for h in heads], 0)
        vrows = np.concatenate([qkv_w[1536 + h * 64:1536 + (h + 1) * 64] for h in heads], 0)
        bq = np.concatenate([qkv_b[h * 64:(h + 1) * 64] for h in heads])
        bk = np.concatenate([qkv_b[768 + h * 64:768 + (h + 1) * 64] for h in heads])
        bv = np.concatenate([qkv_b[1536 + h * 64:1536 + (h + 1) * 64] for h in heads])
        if s == "C":
            # duplicate the single head's q/k rows so qT/kT partitions 64-127
            # mirror 0-63 — lets scores row-tile the two kt-chunks of a slot.
            qrows = np.concatenate([qrows, qrows], 0)
            krows = np.concatenate([krows, krows], 0)
            bq = np.tile(bq, 2)
            bk = np.tile(bk, 2)
        wcols.append(_pack_lhsT(qrows))
        wcols.append(_pack_lhsT(krows))
        wcols.append(_pack_lhsT(vrows))
        wo = np.concatenate([out_w[:, h * 64:(h + 1) * 64].T for h in heads], 0)
        if wo.shape[0] < 128:
            wo = np.concatenate([wo, np.zeros((128 - wo.shape[0], 768), wo.dtype)], 0)
        wcols.append(np.ascontiguousarray(wo).astype(BF16))
        bcols[:len(bq), _BOFF[s]["q"]] = bq
        bcols[:len(bk), _BOFF[s]["k"]] = bk
        bcols[:len(bv), _BOFF[s]["v"]] = bv
    ins["wbuf"] = np.concatenate(wcols, axis=1)
    assert ins["wbuf"].shape == (128, 8832), ins["wbuf"].shape
    ins["bbuf"] = bcols
    return ins


def _build_module():
    from concourse import bacc
    import concourse.mybir as mybir
    import concourse.tile as tile
    from concourse.bass import ts, ds

    dt = mybir.dt
    f32, bf = dt.float32, dt.bfloat16
    EXP = mybir.ActivationFunctionType.Exp
    MULT = mybir.AluOpType.mult

    nc = bacc.Bacc("TRN2", target_bir_lowering=False, debug=False)

    xtd = {s: nc.dram_tensor(f"xt{s}", (4, 128, 6, 512), bf, kind="ExternalInput")
           for s in "ABC"}
    wbuf = nc.dram_tensor("wbuf", (128, 8832), bf, kind="ExternalInput")
    bbuf = nc.dram_tensor("bbuf", (128, 9), f32, kind="ExternalInput")
    # zout is TRANSPOSED: [768 out-dims, 4096 query slots] — the out
    # projection computes z^T = Wo_chunk^T @ osb directly (N=512 matmuls,
    # full K=128), and the host untransposes during unshard.
    zout = nc.dram_tensor("zout", (768, 4096), bf, kind="ExternalOutput")

    with tile.TileContext(nc) as tc:
        with (
            tc.tile_pool(name="const", bufs=1) as constp,
            tc.tile_pool(name="xtp", bufs=2) as xtp,
            tc.tile_pool(name="projsb", bufs=2) as projp,
            tc.tile_pool(name="ptp", bufs=6) as ptp,
            tc.tile_pool(name="osbp", bufs=2) as osbp,
            tc.tile_pool(name="rsbp", bufs=2) as rsbp,
            tc.tile_pool(name="zsbp", bufs=2) as zsbp,
            tc.tile_pool(name="pst", bufs=2, space="PSUM") as pst,
            tc.tile_pool(name="pso", bufs=1, space="PSUM") as pso,
            tc.tile_pool(name="paux", bufs=2, space="PSUM") as paux,
        ):
            wsb = constp.tile([128, 8832], bf)
            bsb = constp.tile([128, 9], f32)
            wtile = constp.tile([128, 512], bf)
            nc.vector.memset(wtile[:], 0.03125)
            # preload the exp table set (~2.7us) during the DMA window
            # instead of at the first real score activation.
            tpre = constp.tile([1, 2], bf)
            nc.scalar.activation(tpre[0:1, 0:1], wtile[0:1, 0:1], EXP,
                                 scale=0.125)
            xts = {}
            for s in "ABC":
                xts[s] = xtp.tile([128, 6, 2048], bf, tag="xt", name=f"xtsb{s}")
            # Input DMA phasing exploits the 8 DMAHW completion lanes: lanes
            # are assigned round-robin over dma_start emission order, and a
            # DMA whose lane is still occupied waits for its predecessor. So
            # the first 8 DMAs are exactly the A-set critical path (tokens +
            # weights + bias, split across both HWDGE rings); the B-set and
            # C-set transfers are emitted as 8 pieces each, so every piece is
            # gated on a prior phase's completion and never steals bandwidth
            # from the critical path.
            for t in range(4):
                nc.sync.dma_start(xts["A"][:, :, ts(t, 512)], xtd["A"][t])
            nc.scalar.dma_start(wsb[:, 768:1536], wbuf[:, 768:1536])
            nc.scalar.dma_start(wsb[:, 0:768], wbuf[:, 0:768])
            nc.scalar.dma_start(wsb[:, 1536:3072], wbuf[:, 1536:3072])
            nc.scalar.dma_start(bsb[:], bbuf[:])
            # Hard phase gate: tiny DVE copies that READ the tail of every
            # phase-1 transfer and then WRITE one element inside each
            # phase-2 destination region. The phase-2 DMAs pick up a WAW
            # dependency on those writes, so their data transfer cannot
            # start until the whole critical phase has landed.
            gate = constp.tile([1, 8], bf)
            for gi, src in enumerate([
                xts["A"][0:1, 5, 511:512], xts["A"][0:1, 5, 1023:1024],
                xts["A"][0:1, 5, 1535:1536], xts["A"][0:1, 5, 2047:2048],
                wsb[0:1, 767:768], wsb[0:1, 1535:1536], wsb[0:1, 3071:3072],
            ]):
                nc.vector.tensor_copy(gate[0:1, gi:gi + 1], src)
            nc.vector.tensor_copy(xts["B"][0:1, 0, 0:1], gate[0:1, 0:1])
            nc.vector.tensor_copy(wsb[0:1, 3072:3073], gate[0:1, 1:2])
            # phase 2: B tokens + B/C weights (8 pieces, lanes 0-7)
            for t in range(4):
                nc.sync.dma_start(xts["B"][:, :, ts(t, 512)], xtd["B"][t])
            nc.sync.dma_start(wsb[:, 3072:4608], wbuf[:, 3072:4608])
            nc.sync.dma_start(wsb[:, 4608:6144], wbuf[:, 4608:6144])
            nc.sync.dma_start(wsb[:, 6144:7680], wbuf[:, 6144:7680])
            nc.sync.dma_start(wsb[:, 7680:8832], wbuf[:, 7680:8832])
            # phase 3: C tokens (xtC shares the A buffer, so its DMA also
            # waits for the A projections to finish reading).
            for t in range(4):
                nc.sync.dma_start(xts["C"][:, :, ts(t, 512)], xtd["C"][t])

            def xt_rhs(s, t):
                """[128, 6, 512] view of token chunk t of set s."""
                return xts[s][:, :, ts(t, 512)]

            wmc = [0]

            def warm_mm(n=1):
                """Keep-warm matmuls off the memset tile; a fresh paux tile
                per call so the pool rotation never serializes them against
                a live projection accumulator."""
                wps = paux.tile([128, 512], f32, tag="aux",
                                name=f"wm{wmc[0]}")
                wmc[0] += 1
                for _ in range(n):
                    nc.tensor.matmul(wps[:], lhsT=wtile[:, 0:128],
                                     rhs=wtile[:], start=True, stop=True)

            def make_proj(s):
                """Returns (qT, kT, vhalves, counters, pre_steps, post_steps).
                q/k projections always produce all 128 partitions (set C's
                weight rows are host-duplicated); v stays at the set's head
                count. Attention for the set can start once pre_steps are done
                (it touches the second v half only from slot kp=4 on)."""
                W, B = _WOFF[s], _BOFF[s]
                two = s != "C"
                m_len = 2048 if s == "A" else 1024
                Mv = 128 if two else 64
                qT = projp.tile([128, m_len], bf, tag="qT", name=f"qT{s}")
                kT = projp.tile([128, 2048], bf, tag="kT", name=f"kT{s}")
                vhalves = [projp.tile([128, 8, 130], bf, tag=f"vsb{h}",
                                      name=f"v{s}{h}") for h in range(2)]

                counters = {"k": 0, "q": 0, "v": 0}

                def qk_step(nm, t):
                    def f():
                        dest = qT if nm == "q" else kT
                        woff, bcol = W[nm], B[nm]
                        ps = paux.tile([128, 512], f32, tag="aux",
                                       name=f"ps{s}{nm}{t}")
                        xt = xt_rhs(s, t)
                        for e in range(6):
                            nc.tensor.matmul(
                                ps[:],
                                lhsT=wsb[:, woff + e * 128: woff + (e + 1) * 128],
                                rhs=xt[:, e, :],
                                start=(e == 0), stop=(e == 5))
                        nc.vector.tensor_scalar_add(
                            dest[:, ts(t, 512)], ps[:], bsb[:, bcol:bcol + 1])
                        counters[nm] += 1
                    return f

                def v_step(tt):
                    def f():
                        vsb = vhalves[tt // 8]
                        ps = paux.tile([128, 512], f32, tag="aux",
                                       name=f"ps{s}v{tt}")
                        xt = xt_rhs(s, tt // 4)
                        for e in range(6):
                            nc.tensor.matmul(
                                ps[:, :Mv], lhsT=xt[:, e, ts(tt % 4, 128)],
                                rhs=wsb[:, W["v"] + e * Mv: W["v"] + (e + 1) * Mv],
                                start=(e == 0), stop=(e == 5))
                        if two:
                            dst = vsb[:, tt % 8, 0:130].rearrange(
                                "p (h w) -> p h w", w=65)[:, :, 0:64]
                            nc.vector.tensor_copy(
                                dst, ps[:, 0:128].rearrange("p (h w) -> p h w", w=64))
                        else:
                            nc.vector.tensor_copy(vsb[:, tt % 8, 0:64],
                                                  ps[:, 0:64])
                        assert counters["v"] == tt, (s, tt, counters)
                        counters["v"] += 1
                    return f

                def ones_step():
                    for vsb in vhalves:
                        nc.vector.memset(vsb[:, :, 64:65], 1.0)
                        if two:
                            nc.vector.memset(vsb[:, :, 129:130], 1.0)

                pre = [(0.05, ones_step)]
                pre += [(1.3, qk_step("k", t)) for t in range(4)]
                pre += [(1.3, qk_step("q", 0))]
                # Post steps are paced into attention slots (split between
                # the previous set's and the set's own; deadlines asserted
                # at emit time). q1 is ordered early so a self-paced set has
                # its mc=1 queries ready by slot 8.
                post = [(0.4, v_step(tt)) for tt in range(6)]
                post += [(1.3, qk_step("q", 1))] if m_len > 512 else []
                post += [(0.4, v_step(tt)) for tt in range(6, 16)]
                post += [(1.3, qk_step("q", t)) for t in range(2, m_len // 512)]
                return qT, kT, vhalves, counters, pre, post

            def attn_set(s, qT, kT, vhalves, counters, zrow0, fillers):
                """Software-pipelined attention for one set. fillers is the
                tail of this set's projection steps followed by the next
                set's; up to two cheap steps pop per slot."""
                W, B = _WOFF[s], _BOFF[s]
                two = s != "C"
                m_len = 2048 if s == "A" else 1024
                nmc = m_len // 512
                heads = [("a", 0)] + ([("b", 64)] if two else [])
                M = 128 if two else 64

                def pace():
                    if not fillers:
                        # heartbeat: keep the PE duty cycle above the HAM
                        # throttle threshold in filler-starved slots.
                        warm_mm()
                        return
                    c, f = fillers.pop(0)
                    f()
                    if c < 0.5 and fillers and fillers[0][0] < 0.5:
                        fillers.pop(0)[1]()

                pts = {}

                def emit_scores(mc, kp):
                    assert counters["q"] >= mc + 1, (s, mc, counters)
                    assert counters["k"] >= min(4, (kp + 1) // 2 + 1), \
                        (s, mc, kp, counters)
                    # One st tile per kt chunk holding BOTH row-tiled halves
                    # (heads a/b, or for set C the even/odd kt pair on the
                    # duplicated rows). The scheduler clusters matmuls by
                    # destination tile, so the concurrent (h0, h64) pair
                    # stays adjacent in the PE queue.
                    if two:
                        for jj in range(2):
                            kt = 2 * kp + jj
                            stj = pst.tile([128, 2, 512], f32, tag="st",
                                           name=f"st{s}{mc}{kp}k{jj}")
                            for hi, (hn, hoff) in enumerate(heads):
                                nc.tensor.matmul(
                                    stj[:, hi],
                                    lhsT=kT[hoff:hoff + 64, ts(kt, 128)],
                                    rhs=qT[hoff:hoff + 64, ts(mc, 512)],
                                    start=True, stop=True)
                            pt = ptp.tile([128, 2, 512], bf, tag="pt",
                                          name=f"pt{s}{mc}{kp}k{jj}")
                            nc.scalar.activation(pt[:], stj[:], EXP,
                                                 scale=0.125)
                            pts[(mc, kp, jj)] = pt
                    else:
                        # single head: pair the slot's two kt chunks on row
                        # groups h0/h64 via the duplicated kT/qT rows.
                        st = pst.tile([128, 2, 512], f32, tag="st",
                                      name=f"st{s}{mc}{kp}")
                        for jj in range(2):
                            kt = 2 * kp + jj
                            hoff = 64 * jj
                            nc.tensor.matmul(
                                st[:, jj],
                                lhsT=kT[hoff:hoff + 64, ts(kt, 128)],
                                rhs=qT[hoff:hoff + 64, ts(mc, 512)],
                                start=True, stop=True)
                        pt = ptp.tile([128, 2, 512], bf, tag="pt",
                                      name=f"pt{s}{mc}{kp}")
                        nc.scalar.activation(pt[:], st[:], EXP, scale=0.125)
                        pts[(mc, kp, 0)] = pt

                def emit_av(o_ps, mc, kp, only=None):
                    assert counters["v"] >= 2 * kp + 2, (s, mc, kp, counters)
                    for hi, (hn, hoff) in enumerate(heads):
                        if only is not None and hn != only:
                            continue
                        voff = 0 if hn == "a" else 65
                        for jj in range(2):
                            kt = 2 * kp + jj
                            vsb = vhalves[kt // 8]
                            if two:
                                # pt tile for kt chunk jj; head slice hi
                                rhs = pts[(mc, kp, jj)][:, hi]
                            else:
                                rhs = pts[(mc, kp, 0)][:, jj]
                            nc.tensor.matmul(
                                o_ps[hn], lhsT=vsb[:, kt % 8, voff:voff + 65],
                                rhs=rhs,
                                start=(kt == 0), stop=(kt == 15),
                                skip_group_check=True)
                        if hn == heads[-1][0]:
                            for jj in (range(2) if two else (0,)):
                                pts.pop((mc, kp, jj), None)

                def emit_recip_head(o_ps, mc, i, hn):
                    # NB: reciprocal_approx_fast reading PSUM directly is
                    # numerically broken on hardware (sim passes) — the D row
                    # must be staged through SBUF first.
                    dsb = rsbp.tile([1, 512], f32, tag=f"dsb_{hn}",
                                    name=f"d{s}{mc}{hn}")
                    rsb = rsbp.tile([1, 512], f32, tag=f"rsb_{hn}",
                                    name=f"r{s}{mc}{hn}")
                    nc.vector.tensor_copy(dsb[0:1, :], o_ps[hn][64:65, :])
                    nc.vector.reciprocal_approx_fast(rsb[0:1, :], dsb[0:1, :])
                    rb = rsbp.tile([64, 512], f32, tag=f"rb_{hn}",
                                   name=f"rb{s}{mc}{hn}")
                    nc.gpsimd.partition_broadcast(rb[:], rsb[0:1, :])
                    return rb

                def emit_av7_recips(o_ps, mc):
                    # per head: final AV pair, then immediately start that
                    # head's 1/D chain (copy+recip on DVE, broadcast on
                    # gpsimd) so it overlaps the other head's AV.
                    rbs = {}
                    for i, (hn, hoff) in enumerate(heads):
                        emit_av(o_ps, mc, 7, only=hn)
                        rbs[hn] = emit_recip_head(o_ps, mc, i, hn)
                    return rbs

                def emit_norm(o_ps, rbs, mc):
                    osb = osbp.tile([128, 512], bf, tag="osb",
                                    name=f"osb{s}{mc}")
                    for hn, hoff in heads:
                        nc.vector.tensor_tensor(
                            osb[hoff:hoff + 64, :], o_ps[hn][0:64, :],
                            rbs[hn][:], MULT)
                    if not two:
                        # out-proj contracts all 128 rows (wo rows 64-127
                        # are zero) — clear the unowned half once.
                        nc.vector.memset(osb[64:128, :], 0.0)
                    zsb = zsbp.tile([128, 6, 512], bf, tag="zsb",
                                    name=f"z{s}{mc}")
                    return osb, zsb

                def emit_outproj_oc(osb, zsb, mc, oc):
                    # z^T chunk: [128 out-dims, 512 queries] = Wo_oc^T @ osb.
                    # Full K=128 (set C's wo rows 64-127 are zero-padded).
                    zp = paux.tile([128, 512], f32, tag="aux",
                                   name=f"zp{s}{mc}{oc}")
                    nc.tensor.matmul(
                        zp[:], lhsT=wsb[:, W["o"] + oc * 128:
                                        W["o"] + (oc + 1) * 128],
                        rhs=osb[:], start=True, stop=True)
                    nc.vector.tensor_copy(zsb[:, oc, :], zp[:])
                    if oc == 5:
                        qbase = zrow0 + mc * 512
                        dst = zout[ds(0, 768), ds(qbase, 512)].rearrange(
                            "(c p) q -> p c q", p=128)
                        nc.sync.dma_start(dst, zsb[:])

                prev = None  # (o_ps, rsb, mc) of previous mc chunk
                pending = None  # (osb, zsb, mc, next_tt)
                o_cur = None
                for mc in range(nmc):
                    o_cur = {hn: pso.tile([65, 512], f32, tag=f"o_{hn}",
                                          name=f"o{s}{mc}{hn}")
                             for hn, _ in heads}
                    for kp in range(8):
                        # scores first: they have no dependency on the
                        # recip/norm chain, so they keep the PE busy through
                        # the kp==1/2 softmax-denominator latency window
                        # (putting them later trips HAM re-throttling).
                        emit_scores(mc, kp)
                        pace()
                        if kp == 0:
                            if prev is not None:
                                emit_av(prev[0], prev[2], 6)
                        elif kp == 1:
                            # AV7 + the 1/D chain, then norm at the END of
                            # the slot: rb lands just after the next slot's
                            # scores, so the kp=2..4 out-proj chunks never
                            # queue the PE behind the recip latency.
                            if prev is not None:
                                rbs = emit_av7_recips(prev[0], prev[2])
                                pace()
                                osb, zsb = emit_norm(prev[0], rbs, prev[2])
                                pending = (osb, zsb, prev[2], 0)
                        else:
                            # spread the out-projection chunks two per slot
                            # (kp=2..4) to level per-slot PE load.
                            if pending is not None:
                                osb, zsb, pmc, oc = pending
                                emit_outproj_oc(osb, zsb, pmc, oc)
                                emit_outproj_oc(osb, zsb, pmc, oc + 1)
                                pending = (osb, zsb, pmc, oc + 2) \
                                    if oc < 4 else None
                            emit_av(o_cur, mc, kp - 2)
                    prev = (o_cur, None, mc)
                # tail: flush the last chunk
                emit_av(prev[0], prev[2], 6)
                rbs = emit_av7_recips(prev[0], prev[2])
                # bridge the recip/norm latency window (fillers if any are
                # left, plus keep-warm heartbeats) so the PE never opens a
                # HAM-visible idle gap at set boundaries or the kernel tail.
                for _ in range(4):
                    pace()
                warm_mm(4)
                warm_mm(4)
                osb, zsb = emit_norm(prev[0], rbs, prev[2])
                for oc in range(6):
                    emit_outproj_oc(osb, zsb, prev[2], oc)

            # Warmup: dummy matmuls off the memset tile (no DMA dependency)
            # start the HAM clock ramp immediately (the result tile is never
            # read). More warm matmuls are woven between the pre-A chains
            # below: the token-chunk landing times jitter run-to-run (all 8
            # cores DMA simultaneously), and the weave keeps the PE duty
            # cycle up through late-landing chunks without delaying early
            # ones by more than a few matmuls.
            warm_mm(5)
            warm_mm(5)

            qTA, kTA, vA, cntA, preA, postA = make_proj("A")
            for pi, (_, f) in enumerate(preA):
                f()
                if pi >= 1:
                    warm_mm(4)
            qTB, kTB, vB, cntB, preB, postB = make_proj("B")
            # One shared filler queue: leftovers carry across sets, and set
            # C hosts the tail of its own projection steps (its slots would
            # otherwise run heartbeats while B's slots were overloaded).
            fillq = postA + preB + postB
            attn_set("A", qTA, kTA, vA, cntA, 0, fillq)
            qTC, kTC, vC, cntC, preC, postC = make_proj("C")
            fillq.extend(preC + postC[:6])
            attn_set("B", qTB, kTB, vB, cntB, 2048, fillq)
            fillq.extend(postC[6:])
            attn_set("C", qTC, kTC, vC, cntC, 3072, fillq)

    nc.compile()
    return nc


def _get_module():
    if "nc" not in _CACHE:
        _CACHE["nc"] = _build_module()
    return _CACHE["nc"]


def _assemble(results, qkv_b, out_w, out_b):
    out = np.zeros((8192, 768), np.float32)
    for c in range(8):
        plan = _core_plan(c)
        z = results[c]["zout"].astype(np.float32).T  # [4096, 768]
        row0 = {"A": 0, "B": 2048, "C": 3072}
        nqs = {"A": 2048, "B": 1024, "C": 1024}
        for s in "ABC":
            # V-bias contribution (P sums to 1 after normalization, so the
            # v-bias passes through attention and the out projection intact).
            heads = plan[s]["heads"]
            cvec = np.zeros((768,), np.float32)
            for h in plan[s]["heads"]:
                bv = qkv_b[1536 + h * 64:1536 + (h + 1) * 64]
                cvec += bv @ out_w[:, h * 64:(h + 1) * 64].T
            zs = z[row0[s]:row0[s] + nqs[s]] + cvec[None, :]
            order = plan[s]["order"][:nqs[s]]
            out[order] += zs
    out += out_b[None, :]
    return out.reshape(1, 8192, 768)


def kernel(x, qkv_w, qkv_b, out_w, out_b, _trace=False):
    x = np.asarray(x, np.float32)
    qkv_w = np.asarray(qkv_w, np.float32)
    qkv_b = np.asarray(qkv_b, np.float32)
    out_w = np.asarray(out_w, np.float32)
    out_b = np.asarray(out_b, np.float32)

    from concourse.bass_utils import run_bass_kernel_spmd

    nc = _get_module()
    in_maps = [_prep_core_inputs(c, x, qkv_w, qkv_b, out_w) for c in range(8)]
    res = run_bass_kernel_spmd(nc, in_maps, core_ids=list(range(8)), trace=_trace)
    out = _assemble(res.results, qkv_b, out_w, out_b)
    if _trace:
        _CACHE["last_result"] = res
    return out
